# revision 2
# baseline (speedup 1.0000x reference)
"""LoRA embedding lookup on 8 Trainium2 NeuronCores.

out = weight[ids] + ((lora_B @ lora_A).T * 2.0)[ids]
    = weight[ids] + (lora_A[:, ids].T @ (lora_B * 2.0).T)

Strategy: token-parallel. Each of the 8 cores owns 2048 of the 16384
tokens. The per-token LoRA-A coefficient columns lora_A[:, ids] are
sliced out on the host (a [8, 2048] tensor per core) so the device
never gathers or transposes them; the only data-dependent DMA is the
base-row gather from the embedding table. Per block of K tiles the
core does one batched indirect gather ([128, K*1024] rows), K tiny
[8,128]x[8,512] matmuls per 512-column half, a PSUM+base add on
VectorE, and one batched store. No collectives needed.
"""

import numpy as np

import concourse.bacc as bacc
import concourse.bass as bass
import concourse.mybir as mybir
import concourse.tile as tile
from concourse.bass_utils import run_bass_kernel_spmd

VOCAB = 128000
D = 1024
R = 8
SCALING = 2.0
N_CORES = 8
P = 128
CHUNK = 512  # matmul free-dim / PSUM bank size in f32

# test/dev harness can inject extra kwargs and read back results
_RUN_KWARGS: dict = {}
LAST_RESULT = None

# tuning knobs (dev.py overrides via build_nc args)
GATHER_K = 4
TABLE_DT = mybir.dt.float32


def build_nc(ntiles: int, repeat: int = 1, gather_k: int = GATHER_K,
             table_dt=TABLE_DT, g_bufs: int = 3, o_bufs: int = 3):
    """Per-core SPMD graph: gather+LoRA for ntiles*128 tokens.

    repeat>1 re-runs the whole pipeline (same ids, same outputs) for
    within-NEFF timing amplification; results are unchanged.
    """
    assert ntiles % gather_k == 0
    nblocks = ntiles // gather_k
    nc = bacc.Bacc(None, target_bir_lowering=False, debug=False)

    w = nc.dram_tensor("w", [VOCAB, D], table_dt, kind="ExternalInput")
    at = nc.dram_tensor("at", [R, ntiles * P], mybir.dt.float32, kind="ExternalInput")
    bst = nc.dram_tensor("bst", [R, D], mybir.dt.float32, kind="ExternalInput")
    ids = nc.dram_tensor("ids", [P, ntiles], mybir.dt.int32, kind="ExternalInput")
    out = nc.dram_tensor("out", [ntiles * P, D], mybir.dt.float32, kind="ExternalOutput")
    # partition-major view: out_r[p, i, :] == out[i*128 + p, :]
    out_r = out.rearrange("(n p) d -> p n d", p=P)

    with tile.TileContext(nc) as tc:
        with (
            tc.tile_pool(name="const", bufs=1) as const_pool,
            tc.tile_pool(name="gather", bufs=g_bufs) as gpool,
            tc.tile_pool(name="outp", bufs=o_bufs) as opool,
            tc.tile_pool(name="psum_mm", bufs=8, space="PSUM") as psum_mm,
        ):
            ids_tile = const_pool.tile([P, ntiles], mybir.dt.int32)
            nc.sync.dma_start(out=ids_tile[:], in_=ids[:])
            bst_tile = const_pool.tile([R, D], mybir.dt.float32)
            nc.sync.dma_start(out=bst_tile[:], in_=bst[:])
            at_tile = const_pool.tile([R, ntiles * P], mybir.dt.float32)
            nc.sync.dma_start(out=at_tile[:], in_=at[:])

            for g in [b for _ in range(repeat) for b in range(nblocks)]:
                gtile = gpool.tile([P, gather_k * D], table_dt, tag="g")
                nc.gpsimd.indirect_dma_start(
                    out=gtile[:],
                    out_offset=None,
                    in_=w[:],
                    in_offset=bass.IndirectOffsetOnAxis(
                        ap=ids_tile[:, g * gather_k : (g + 1) * gather_k], axis=0
                    ),
                )
                otile = opool.tile([P, gather_k * D], mybir.dt.float32, tag="o")
                for j in range(gather_k):
                    i = g * gather_k + j
                    for h in range(0, D, CHUNK):
                        dp = psum_mm.tile([P, CHUNK], mybir.dt.float32, tag="dp")
                        nc.tensor.matmul(
                            dp[:],
                            at_tile[:, i * P : (i + 1) * P],
                            bst_tile[:, h : h + CHUNK],
                            start=True,
                            stop=True,
                        )
                        nc.vector.tensor_add(
                            out=otile[:, j * D + h : j * D + h + CHUNK],
                            in0=gtile[:, j * D + h : j * D + h + CHUNK],
                            in1=dp[:],
                        )
                nc.sync.dma_start(
                    out=out_r[:, g * gather_k : (g + 1) * gather_k, :],
                    in_=otile[:],
                )

    nc.compile()
    return nc


def _prep_inputs(input_ids, weight, lora_A, lora_B, table_dt=TABLE_DT):
    ids = np.ascontiguousarray(np.asarray(input_ids).reshape(-1).astype(np.int32))
    np_dt = mybir.dt.np(table_dt)
    w = np.asarray(weight, dtype=np.float32)
    if np_dt != np.float32:
        w = w.astype(np_dt)
    a_tok = np.ascontiguousarray(np.asarray(lora_A, dtype=np.float32)[:, ids])  # [8, ntok]
    bst = np.ascontiguousarray(np.asarray(lora_B, dtype=np.float32).T * SCALING)
    return ids, w, a_tok, bst


def kernel(input_ids, weight, lora_A, lora_B):
    global LAST_RESULT
    ids, w, a_tok, bst = _prep_inputs(input_ids, weight, lora_A, lora_B)
    ntok = ids.size
    assert ntok % (N_CORES * P) == 0
    tpc = ntok // N_CORES
    ntiles = tpc // P

    nc = build_nc(ntiles)

    in_maps = []
    for c in range(N_CORES):
        ids_c = ids[c * tpc : (c + 1) * tpc].reshape(ntiles, P).T
        at_c = a_tok[:, c * tpc : (c + 1) * tpc]
        in_maps.append(
            {
                "w": w,
                "at": np.ascontiguousarray(at_c),
                "bst": bst,
                "ids": np.ascontiguousarray(ids_c),
            }
        )

    res = run_bass_kernel_spmd(nc, in_maps, list(range(N_CORES)), **_RUN_KWARGS)
    LAST_RESULT = res
    outs = [res.results[c]["out"] for c in range(N_CORES)]
    full = np.concatenate(outs, axis=0)
    return full.reshape(*np.asarray(input_ids).shape, D).astype(np.float32)
